# revision 47
# baseline (speedup 1.0000x reference)
"""GCN graph-classification kernel for 8 Trainium2 NeuronCores.

Model (PyG-style GCNConv x2 + mean pool + log_softmax):
    h   = x @ W1
    H1  = relu(Ahat @ h + b1)          Ahat = D^-1/2 (A + I) D^-1/2
    H2  = Ahat @ (H1 @ W2) + b2
    out = log_softmax(mean-pool-per-graph(H2))

Distribution strategy (8 cores):
  * nodes dealt to cores round-robin by global degree rank, so per-(core,
    tile) in-degree totals match across cores (SPMD chunk counts tight).
  * layer 1: h = (dis*x) @ W1 computed locally in bf16, AllGathered (bf16,
    Shared-space output); each core aggregates its own nodes' in-edges
    with 64 small dma_gathers spread round-robin over all 4 SWDGE queues
    (queues map to Q7 core pairs, so 4 descriptor generators run
    concurrently - descriptor generation at ~9ns/row/pair is the
    throughput limit, not DMA bandwidth) + one-hot selector matmuls
    (bf16, selectors built 13-at-a-time with one 3D-broadcast is_equal)
    accumulating in PSUM.  Self-loop terms never touch the edge stream:
    each tile's PSUM is seeded with I @ h_own_tile.
  * layer 2 + pooling folded:  pooled = (Q @ H1) @ W2 + b2  with
    Q = P_mean @ Ahat  (500 x 50000, built dense-per-node-tile on host).
    Each core contracts its own H1 tiles against its Q blocks, projects
    the partial per-graph sums through W2 BEFORE reducing, so the
    AllReduce carries only 16x512 floats -> +b2 -> log_softmax.
  All symmetric-norm factors, mean-pool counts and the permutation are
  folded into host-built index/selector/Q arrays (pure index-side prep).
"""

import os
import numpy as np

import concourse.bacc as bacc
import concourse.mybir as mybir
from concourse import tile
from concourse.bass_utils import run_bass_kernel_spmd

# ---------------------------------------------------------------- constants
N, E, F, HID, C, G = 50000, 600000, 128, 128, 16, 500
P = 8                      # NeuronCores
NV = N // P                # nodes per core
NT = (NV + 127) // 128     # node tiles per core (49)
TPAD = NT * 128            # padded per-core node count (6272)
GP = 512                   # padded graph count
GT = GP // 128             # graph tiles
HALF = N // 2              # gather-table half size (int16 index limit)
NB = 32                    # layer-1 gather batches

AF = mybir.ActivationFunctionType
ALU = mybir.AluOpType

LAST_EXEC_NS = None
LAST_RESULT = None


def _install_profile_hook():
    """The agent image's antenv lacks axon_hooks; shim it so
    run_bass_kernel_spmd(trace=True) can capture NTFF profiles."""
    import sys
    import types
    if "antenv.axon_hooks" in sys.modules:
        return True
    try:
        from trn_agent_boot.trn_boot import _ntff_profile_via_ctypes
        hook = _ntff_profile_via_ctypes("/opt/axon/libaxon_pjrt.so")
        if hook is None:
            return False
        mod = types.ModuleType("antenv.axon_hooks")
        mod._hook = hook
        mod.get_axon_ntff_profile_hook = lambda: mod._hook

        def _set(h):
            mod._hook = h
        mod.set_axon_ntff_profile_hook = _set
        sys.modules["antenv.axon_hooks"] = mod
        import antenv
        antenv.axon_hooks = mod
        return True
    except Exception as e:  # profiling is best-effort
        print(f"profile hook unavailable: {e}")
        return False


# ---------------------------------------------------------------- host prep
def _preprocess(x, W1, b1, W2, b2, edge_src, edge_dst, batch):
    import ml_dtypes
    f32 = np.float32
    bf16 = ml_dtypes.bfloat16
    src = np.asarray(edge_src, np.int64)
    dst = np.asarray(edge_dst, np.int64)
    bat = np.asarray(batch, np.int64)
    x = np.asarray(x, f32)

    deg = np.bincount(dst, minlength=N).astype(np.float64) + 1.0
    dis = 1.0 / np.sqrt(deg)
    cnt = np.maximum(np.bincount(bat, minlength=G), 1).astype(np.float64)

    # globally degree-balanced node->core assignment: sort all nodes by
    # degree, deal them round-robin to cores; per-core position = deal rank.
    # Per-(core,tile) in-degree sums become nearly identical across cores,
    # so the max-over-cores chunk padding almost vanishes.
    grank = np.argsort(-deg, kind="stable")    # node ids, degree-descending
    core_of = np.empty(N, np.int64)
    pos = np.empty(N, np.int64)
    order = np.empty(N, np.int64)      # order[k*NV+j] = node at position j
    core_of[grank] = np.arange(N) % P
    pos[grank] = np.arange(N) // P
    order[core_of * NV + pos] = np.arange(N)
    slot = core_of * NV + pos          # row of node in AllGathered h

    # ---- layer-1 edges (self-loops handled separately), grouped
    #      (core, tile, src-half)
    d_own = core_of[dst]
    d_pos = pos[dst]
    t_of = d_pos // 128
    dloc_v = (d_pos % 128).astype(f32)
    sslot = slot[src]
    is_hi = (sslot >= HALF).astype(np.int64)
    idx_v = (sslot - is_hi * HALF).astype(np.int16)

    key = (d_own * NT + t_of) * 2 + is_hi
    ordr = np.argsort(key, kind="stable")
    idx_s = idx_v[ordr]
    dloc_s = dloc_v[ordr]
    bounds = np.searchsorted(key[ordr], np.arange(P * NT * 2 + 1))
    cnts = np.diff(bounds).reshape(P, NT, 2)
    CH = -(-cnts // 128)               # chunks per (core, tile, half)
    CH = CH.max(axis=0)                # [NT, 2]  uniform across cores

    # batches: stride-interleaved tiles so per-batch work is balanced
    tiles_of_batch = [[t for t in range(NT) if t % NB == b] for b in range(NB)]

    # chunk-column / gather-index layout (shared by all cores)
    # gather order per batch: [lo chunks tile-major][hi chunks tile-major]
    # dloc column order per batch: tile-major, each tile [lo chunks][hi]
    # (so one batched is_equal per tile builds all its selectors)
    batch_meta = []       # per batch: dict(nlo, nhi, icol_lo, icol_hi)
    tile_chunks = {}      # t -> [(half, dloc_col, gather_j)]
    tile_colbase = {}     # t -> first dloc column
    col = 0
    icol = 0
    for b in range(NB):
        nlo = int(sum(CH[t, 0] for t in tiles_of_batch[b]))
        nhi = int(sum(CH[t, 1] for t in tiles_of_batch[b]))
        batch_meta.append(dict(nlo=nlo, nhi=nhi, icol_lo=icol,
                               icol_hi=icol + nlo * 8))
        jlo = jhi = 0
        for t in tiles_of_batch[b]:
            tile_colbase[t] = col
            lst = []
            for _ in range(int(CH[t, 0])):
                lst.append((0, col, jlo))
                col += 1
                jlo += 1
            for _ in range(int(CH[t, 1])):
                lst.append((1, col, jhi))
                col += 1
                jhi += 1
            tile_chunks[t] = lst
        icol += (nlo + nhi) * 8
    NCH = col
    NIDX = NCH * 128
    if os.environ.get("GCN_DEBUG"):
        print(f"NCH={NCH} NIDX={NIDX} real_msgs={cnts.sum(axis=(1,2)).max()}")

    # per-core data arrays
    xT = np.zeros((P, 128, TPAD), bf16)
    disc = np.zeros((P, 128, NT), f32)
    qb = np.zeros((P, TPAD, GP), f32)
    dloc_all = np.full((P, 128, NCH), -1.0, f32)
    idx_flat = np.zeros((P, NIDX), np.int16)

    for k in range(P):
        ok = order[k * NV:(k + 1) * NV]
        # dis[src] prescale folded into x so phase B is a pure matmul
        xT[k, :, :NV] = (x[ok] * dis[ok][:, None]).T.astype(bf16)
        d = np.zeros(TPAD, f32)
        d[:NV] = dis[ok].astype(f32)
        disc[k] = d.reshape(NT, 128).T

    # fill chunk idx / dloc tables
    for b in range(NB):
        m = batch_meta[b]
        for h, base_icol in ((0, m["icol_lo"]), (1, m["icol_hi"])):
            jh = 0
            for t in tiles_of_batch[b]:
                nchunk = int(CH[t, h])
                if nchunk > 0:
                    dcolbase = tile_colbase[t] + (0 if h == 0
                                                  else int(CH[t, 0]))
                    for k in range(P):
                        gi = (k * NT + t) * 2 + h
                        g0, g1 = bounds[gi], bounds[gi + 1]
                        n = g1 - g0
                        fbase = base_icol * 16 + jh * 128
                        idx_flat[k, fbase:fbase + n] = idx_s[g0:g1]
                        pp = np.arange(n) % 128
                        cc = np.arange(n) // 128
                        dloc_all[k, pp, dcolbase + cc] = dloc_s[g0:g1]
                jh += nchunk
    assert idx_flat.min() >= 0 and int(idx_flat.max()) < HALF
    # wrap gather indices: i -> [i % 16, i // 16], replicated to 128 partitions
    idxs = np.tile(
        idx_flat.reshape(P, NIDX // 16, 16).transpose(0, 2, 1), (1, 8, 1)
    ).astype(np.int16)

    # ---- layer-2 Q blocks: qb[core, pos[src], g] += norm/cnt[g]
    #      (self-loops included here)
    e_src = np.concatenate([src, np.arange(N)])
    e_dst = np.concatenate([dst, np.arange(N)])
    g_of = bat[e_dst]
    val = (dis[e_src] * dis[e_dst] / cnt[g_of]).astype(f32)
    np.add.at(qb, (core_of[e_src], pos[e_src], g_of), val)

    sum8 = np.zeros((128, 16), np.float32)
    sum8[np.arange(128), np.arange(128) % 16] = 1.0

    iota2d = np.broadcast_to(
        np.arange(128, dtype=f32), (128, 128)).astype(bf16).copy()
    eye128 = np.eye(128, dtype=f32).astype(bf16)
    eye16 = np.eye(16, dtype=f32)

    qb = qb.astype(bf16)
    dloc_bf = dloc_all.astype(bf16)

    W1 = np.ascontiguousarray(np.asarray(W1, f32).astype(bf16))
    W2 = np.ascontiguousarray(np.asarray(W2, f32))
    b1 = np.asarray(b1, f32)
    b2 = np.asarray(b2, f32)
    use_b1 = bool(np.any(b1))
    use_b2 = bool(np.any(b2))

    in_maps = []
    for k in range(P):
        m = {
            "xT": np.ascontiguousarray(xT[k]),
            "qb": np.ascontiguousarray(qb[k]),
            "idxs": np.ascontiguousarray(idxs[k]),
            "dloc": np.ascontiguousarray(dloc_bf[k]),
            "disc": np.ascontiguousarray(disc[k]),
            "w1": W1, "w2": W2, "sum8": sum8,
            "iota": iota2d, "eye128": eye128, "eye16": eye16,
        }
        if use_b1:
            rr = np.zeros((1, TPAD), f32)
            rr[0, :NV] = np.sqrt(deg[order[k * NV:(k + 1) * NV]]).astype(f32)
            m["rdis"] = rr
            m["b1r"] = b1.reshape(1, F)
        if use_b2:
            m["b2r"] = b2.reshape(C, 1)
        in_maps.append(m)

    plan = dict(NCH=NCH, NIDX=NIDX, CH=CH, tiles_of_batch=tiles_of_batch,
                tile_chunks=tile_chunks, tile_colbase=tile_colbase,
                batch_meta=batch_meta, use_b1=use_b1, use_b2=use_b2)
    return plan, in_maps


# ---------------------------------------------------------------- bass build
def _build(plan):
    dt = mybir.dt
    f32, bf16, i16 = dt.float32, dt.bfloat16, dt.int16
    NCH, NIDX = plan["NCH"], plan["NIDX"]
    use_b1, use_b2 = plan["use_b1"], plan["use_b2"]

    nc = bacc.Bacc("TRN2", target_bir_lowering=False, debug=False,
                   num_devices=P, num_swdge_queues=4)
    xT_d = nc.dram_tensor("xT", [128, TPAD], bf16, kind="ExternalInput")
    qb_d = nc.dram_tensor("qb", [TPAD, GP], bf16, kind="ExternalInput")
    idxs_d = nc.dram_tensor("idxs", [128, NIDX // 16], i16, kind="ExternalInput")
    dloc_d = nc.dram_tensor("dloc", [128, NCH], bf16, kind="ExternalInput")
    disc_d = nc.dram_tensor("disc", [128, NT], f32, kind="ExternalInput")
    w1_d = nc.dram_tensor("w1", [F, HID], bf16, kind="ExternalInput")
    w2_d = nc.dram_tensor("w2", [HID, C], f32, kind="ExternalInput")
    iota_d = nc.dram_tensor("iota", [128, 128], bf16, kind="ExternalInput")
    eye128_d = nc.dram_tensor("eye128", [128, 128], bf16, kind="ExternalInput")
    eye_d = nc.dram_tensor("eye16", [16, 16], f32, kind="ExternalInput")
    sum8_d = nc.dram_tensor("sum8", [128, 16], f32, kind="ExternalInput")
    if use_b1:
        rdis_d = nc.dram_tensor("rdis", [1, TPAD], f32, kind="ExternalInput")
        b1_d = nc.dram_tensor("b1r", [1, F], f32, kind="ExternalInput")
    if use_b2:
        b2_d = nc.dram_tensor("b2r", [C, 1], f32, kind="ExternalInput")
    y_d = nc.dram_tensor("y", [G, C], f32, kind="ExternalOutput")
    shared = bool(int(os.environ.get("GCN_SHARED", "1")))
    # AllGather output in the Shared scratchpad (faster HBM-HBM collective)
    h_full = nc.dram_tensor("h_full_sh", [N, F], bf16, kind="Internal",
                            addr_space="Shared" if shared else "Local")

    with tile.TileContext(nc) as tc:
        cpool = tc.alloc_tile_pool(name="const", bufs=1)
        dram = tc.alloc_tile_pool(name="dram", bufs=1, space="DRAM")

        w1_sb = cpool.tile([F, HID], bf16)
        nc.sync.dma_start(w1_sb[:], w1_d[:, :])
        w2_sb = cpool.tile([HID, C], f32)
        nc.sync.dma_start(w2_sb[:], w2_d[:, :])
        disc_sb = cpool.tile([128, NT], f32)
        nc.sync.dma_start(disc_sb[:], disc_d[:, :])
        iota_sb = cpool.tile([128, 128], bf16)
        nc.sync.dma_start(iota_sb[:], iota_d[:, :])
        eye128_sb = cpool.tile([128, 128], bf16)
        nc.sync.dma_start(eye128_sb[:], eye128_d[:, :])
        eye_sb = cpool.tile([16, 16], f32)
        nc.sync.dma_start(eye_sb[:], eye_d[:, :])
        sum8_sb = cpool.tile([128, 16], f32)
        nc.sync.dma_start(sum8_sb[:], sum8_d[:, :])
        idxs_sb = cpool.tile([128, NIDX // 16], i16)
        nc.sync.dma_start(idxs_sb[:], idxs_d[:, :])
        dloc_sb = cpool.tile([128, NCH], bf16)
        nc.sync.dma_start(dloc_sb[:], dloc_d[:, :])
        h1_sb = cpool.tile([128, TPAD], bf16)
        hb_sb = cpool.tile([128, TPAD], bf16)   # own h tiles (self-loop seed)
        if use_b1:
            rdis_sb = cpool.tile([1, TPAD], f32)
            nc.sync.dma_start(rdis_sb[:], rdis_d[:, :])
            b1_sb = cpool.tile([1, F], f32)
            nc.sync.dma_start(b1_sb[:], b1_d[:, :])
        if use_b2:
            b2c_sb = cpool.tile([C, 1], f32)
            nc.sync.dma_start(b2c_sb[:], b2_d[:, :])

        h_own = dram.tile([NV, F], bf16)
        ar_in = dram.tile([16, GP], f32)
        ar_out = dram.tile([16, GP], f32)

        # ---------------- phase B: h = (dis*x) @ W1, AllGather
        # dis prescale folded into xT on host; 4 tiles share a PSUM bank so
        # one activation Copy moves 512 columns at a time.
        with (
            tc.tile_pool(name="xw", bufs=1) as xw,
            tc.tile_pool(name="hp", bufs=2, space="PSUM") as hp,
        ):
            xT_sb = xw.tile([128, TPAD], bf16)
            # chunked load so the first matmuls start early
            XC = TPAD // 8
            for ci in range(8):
                nc.sync.dma_start(xT_sb[:, ci * XC:(ci + 1) * XC],
                                  xT_d[:, ci * XC:(ci + 1) * XC])
            for g0 in range(0, NT, 4):
                gn = min(4, NT - g0)
                ps = hp.tile([128, gn * 128], f32)
                for j in range(gn):
                    t = g0 + j
                    nc.tensor.matmul(ps[:, j * 128:(j + 1) * 128],
                                     lhsT=xT_sb[:, t * 128:(t + 1) * 128],
                                     rhs=w1_sb[:], start=True, stop=True)
                nc.scalar.activation(
                    hb_sb[:, g0 * 128:(g0 + gn) * 128], ps[:], AF.Copy)
                for j in range(gn):
                    t = g0 + j
                    rows = min(128, NV - t * 128)
                    if rows > 0:
                        nc.sync.dma_start(
                            h_own[t * 128:t * 128 + rows, :],
                            hb_sb[0:rows, t * 128:(t + 1) * 128])

        nc.gpsimd.collective_compute(
            "AllGather", ALU.bypass, replica_groups=[list(range(P))],
            ins=[h_own[:].opt()], outs=[h_full[:, :].opt()])

        # ---------------- phase C: layer-1 aggregation + layer-2 contraction
        with tc.tile_pool(name="ptp", bufs=1, space="PSUM") as ptp:
            poolT = ptp.tile([128, GP], f32)
            i_l2 = 0
            with (
                tc.tile_pool(name="glo", bufs=4) as glo_p,
                tc.tile_pool(name="ghi", bufs=4) as ghi_p,
                tc.tile_pool(name="selp", bufs=16) as selp,
                tc.tile_pool(name="qp", bufs=6) as qp,
                tc.tile_pool(name="aggp", bufs=7, space="PSUM") as aggp,
            ):
                for b in range(NB):
                    m = plan["batch_meta"][b]
                    nlo, nhi = m["nlo"], m["nhi"]
                    glo = ghi = None
                    # queues map to Q7 core pairs; 4 queues run 4
                    # descriptor generators concurrently
                    if nlo:
                        glo = glo_p.tile([128, nlo, 128], bf16, tag="glo")
                        nc.gpsimd.dma_gather(
                            out_ap=glo[:], in_ap=h_full[0:HALF, :],
                            idxs_ap=idxs_sb[:, m["icol_lo"]:
                                            m["icol_lo"] + nlo * 8],
                            num_idxs=nlo * 128, num_idxs_reg=nlo * 128,
                            elem_size=F, single_packet=False,
                            queue_num=(2 * b) % 4)
                    if nhi:
                        ghi = ghi_p.tile([128, nhi, 128], bf16, tag="ghi")
                        nc.gpsimd.dma_gather(
                            out_ap=ghi[:], in_ap=h_full[HALF:N, :],
                            idxs_ap=idxs_sb[:, m["icol_hi"]:
                                            m["icol_hi"] + nhi * 8],
                            num_idxs=nhi * 128, num_idxs_reg=nhi * 128,
                            elem_size=F, single_packet=False,
                            queue_num=(2 * b + 1) % 4)
                    for t in plan["tiles_of_batch"][b]:
                        chunks = plan["tile_chunks"][t]
                        K = len(chunks)
                        c0 = plan["tile_colbase"][t]
                        ps = aggp.tile([128, 128], f32, tag="agg")
                        # self-loop seed: I @ h_own_tile (PE-ordered with the
                        # accumulating chunk matmuls -- deterministic)
                        nc.tensor.matmul(
                            ps[:], lhsT=eye128_sb[:],
                            rhs=hb_sb[:, t * 128:(t + 1) * 128],
                            start=True, stop=(K == 0 and not use_b1))
                        if use_b1:
                            nc.tensor.matmul(
                                ps[:], lhsT=rdis_sb[0:1, t * 128:(t + 1) * 128],
                                rhs=b1_sb[:], start=False, stop=(K == 0))
                        if K:
                            sel = selp.tile([128, K * 128], bf16, tag="sel")
                            if bool(int(os.environ.get("GCN_SEL3D", "1"))):
                                nc.vector.tensor_tensor(
                                    out=sel[:].rearrange("p (k f) -> p k f",
                                                         k=K),
                                    in0=iota_sb[:].unsqueeze(1).broadcast_to(
                                        [128, K, 128]),
                                    in1=dloc_sb[:, c0:c0 + K].unsqueeze(2)
                                        .broadcast_to([128, K, 128]),
                                    op=ALU.is_equal)
                            else:
                                for ci in range(K):
                                    nc.vector.tensor_tensor(
                                        out=sel[:, ci * 128:(ci + 1) * 128],
                                        in0=iota_sb[:],
                                        in1=dloc_sb[:, c0 + ci:c0 + ci + 1]
                                            .to_broadcast([128, 128]),
                                        op=ALU.is_equal)
                        for ci, (h, dcol, j) in enumerate(chunks):
                            gsrc = ghi if h else glo
                            nc.tensor.matmul(
                                ps[:], lhsT=sel[:, ci * 128:(ci + 1) * 128],
                                rhs=gsrc[:, j, :],
                                start=False, stop=(ci == K - 1))
                        nc.scalar.activation(
                            h1_sb[:, t * 128:(t + 1) * 128], ps[:], AF.Relu,
                            scale=disc_sb[:, t:t + 1])
                        # layer 2: poolT += H1_tile^T-contraction with Q block
                        qt = qp.tile([128, GP], bf16, tag="q")
                        nc.sync.dma_start(
                            qt[:], qb_d[t * 128:(t + 1) * 128, :])
                        nc.tensor.matmul(
                            poolT[:],
                            lhsT=h1_sb[:, t * 128:(t + 1) * 128],
                            rhs=qt[:],
                            start=(i_l2 == 0), stop=(i_l2 == NT - 1))
                        i_l2 += 1

            # project through W2 BEFORE the AllReduce: 256KB -> 32KB payload
            pt_sb = cpool.tile([128, GP], f32)
            nc.scalar.activation(pt_sb[:], poolT[:], AF.Copy)
            with tc.tile_pool(name="w2p", bufs=1, space="PSUM") as w2p:
                out2 = w2p.tile([16, GP], f32)
                nc.tensor.matmul(out2[:], lhsT=w2_sb[:], rhs=pt_sb[:],
                                 start=True, stop=True)
                o2_sb = cpool.tile([16, GP], f32)
                nc.scalar.activation(o2_sb[:], out2[:], AF.Copy)
            nc.sync.dma_start(ar_in[:], o2_sb[:])

        nc.gpsimd.collective_compute(
            "AllReduce", ALU.add, replica_groups=[list(range(P))],
            ins=[ar_in[:].opt()], outs=[ar_out[:].opt()])

        # ---------------- phase D: bias, log_softmax
        with (
            tc.tile_pool(name="fin", bufs=1) as fin,
            tc.tile_pool(name="fps", bufs=2, space="PSUM") as fps,
            tc.tile_pool(name="sm", bufs=4) as smp,
        ):
            logitsT = fin.tile([16, GP], f32)
            nc.sync.dma_start(logitsT[:], ar_out[:])
            if use_b2:
                # bias along classes = per-partition scalar [16, 1]
                nc.vector.tensor_tensor(
                    out=logitsT[:], in0=logitsT[:],
                    in1=b2c_sb[:, 0:1].to_broadcast([16, GP]), op=ALU.add)
            for gt in range(min(GT, -(-G // 128))):
                tp = fps.tile([128, 16], f32, tag="tp")
                nc.tensor.transpose(
                    tp[:], logitsT[:, gt * 128:(gt + 1) * 128], eye_sb[:])
                nmx = smp.tile([128, 1], f32, tag="nmx")
                nc.vector.reduce_max(out=nmx[:], in_=tp[:],
                                     axis=mybir.AxisListType.X, negate=True)
                ex = smp.tile([128, 16], f32, tag="ex")
                nc.scalar.activation(ex[:], tp[:], AF.Exp, bias=nmx[:, 0:1])
                sm = smp.tile([128, 1], f32, tag="sm")
                nc.vector.reduce_sum(out=sm[:], in_=ex[:],
                                     axis=mybir.AxisListType.X)
                lse = smp.tile([128, 1], f32, tag="lse")
                nc.scalar.activation(lse[:], sm[:], AF.Ln)
                res = smp.tile([128, 16], f32, tag="res")
                nc.vector.tensor_scalar(res[:], tp[:], nmx[:, 0:1],
                                        lse[:, 0:1], ALU.add, ALU.subtract)
                rows = min(128, G - gt * 128)
                nc.sync.dma_start(y_d[gt * 128:gt * 128 + rows, :],
                                  res[0:rows, :])
        dram.release()
        cpool.release()
    nc.compile()
    return nc


# ---------------------------------------------------------------- entry
def kernel(x, W1, b1, W2, b2, edge_src, edge_dst, batch):
    global LAST_EXEC_NS, LAST_RESULT
    plan, in_maps = _preprocess(x, W1, b1, W2, b2,
                                edge_src, edge_dst, batch)
    nc = _build(plan)
    trace = bool(int(os.environ.get("GCN_TRACE", "0")))
    kw = {}
    if trace and _install_profile_hook():
        kw = dict(trace=True, trace_cores=[0])
    res = run_bass_kernel_spmd(nc, in_maps, core_ids=list(range(P)), **kw)
    LAST_RESULT = res
    LAST_EXEC_NS = res.exec_time_ns
    return np.ascontiguousarray(res.results[0]["y"].astype(np.float32))


# revision 48
# speedup vs baseline: 1.0176x; 1.0176x over previous
"""GCN graph-classification kernel for 8 Trainium2 NeuronCores.

Model (PyG-style GCNConv x2 + mean pool + log_softmax):
    h   = x @ W1
    H1  = relu(Ahat @ h + b1)          Ahat = D^-1/2 (A + I) D^-1/2
    H2  = Ahat @ (H1 @ W2) + b2
    out = log_softmax(mean-pool-per-graph(H2))

Distribution strategy (8 cores):
  * nodes dealt to cores round-robin by global degree rank, so per-(core,
    tile) in-degree totals match across cores (SPMD chunk counts tight).
  * layer 1: h = (dis*x) @ W1 computed locally in bf16, AllGathered (bf16,
    Shared-space output); each core aggregates its own nodes' in-edges
    with 64 small dma_gathers spread round-robin over all 4 SWDGE queues
    (queues map to Q7 core pairs, so 4 descriptor generators run
    concurrently - descriptor generation at ~9ns/row/pair is the
    throughput limit, not DMA bandwidth) + one-hot selector matmuls
    (bf16, selectors built 13-at-a-time with one 3D-broadcast is_equal)
    accumulating in PSUM.  Self-loop terms never touch the edge stream:
    each tile's PSUM is seeded with I @ h_own_tile.
  * layer 2 + pooling folded:  pooled = (Q @ H1) @ W2 + b2  with
    Q = P_mean @ Ahat  (500 x 50000, built dense-per-node-tile on host).
    Each core contracts its own H1 tiles against its Q blocks, projects
    the partial per-graph sums through W2 BEFORE reducing, so the
    AllReduce carries only 16x512 floats -> +b2 -> log_softmax.
  All symmetric-norm factors, mean-pool counts and the permutation are
  folded into host-built index/selector/Q arrays (pure index-side prep).
"""

import os
import numpy as np

import concourse.bacc as bacc
import concourse.mybir as mybir
from concourse import tile
from concourse.bass_utils import run_bass_kernel_spmd

# ---------------------------------------------------------------- constants
N, E, F, HID, C, G = 50000, 600000, 128, 128, 16, 500
P = 8                      # NeuronCores
NV = N // P                # nodes per core
NT = (NV + 127) // 128     # node tiles per core (49)
TPAD = NT * 128            # padded per-core node count (6272)
GP = 512                   # padded graph count
GT = GP // 128             # graph tiles
HALF = N // 2              # gather-table half size (int16 index limit)
NB = 32                    # layer-1 gather batches

AF = mybir.ActivationFunctionType
ALU = mybir.AluOpType

LAST_EXEC_NS = None
LAST_RESULT = None


def _install_profile_hook():
    """The agent image's antenv lacks axon_hooks; shim it so
    run_bass_kernel_spmd(trace=True) can capture NTFF profiles."""
    import sys
    import types
    if "antenv.axon_hooks" in sys.modules:
        return True
    try:
        from trn_agent_boot.trn_boot import _ntff_profile_via_ctypes
        hook = _ntff_profile_via_ctypes("/opt/axon/libaxon_pjrt.so")
        if hook is None:
            return False
        mod = types.ModuleType("antenv.axon_hooks")
        mod._hook = hook
        mod.get_axon_ntff_profile_hook = lambda: mod._hook

        def _set(h):
            mod._hook = h
        mod.set_axon_ntff_profile_hook = _set
        sys.modules["antenv.axon_hooks"] = mod
        import antenv
        antenv.axon_hooks = mod
        return True
    except Exception as e:  # profiling is best-effort
        print(f"profile hook unavailable: {e}")
        return False


# ---------------------------------------------------------------- host prep
def _preprocess(x, W1, b1, W2, b2, edge_src, edge_dst, batch):
    import ml_dtypes
    f32 = np.float32
    bf16 = ml_dtypes.bfloat16
    src = np.asarray(edge_src, np.int64)
    dst = np.asarray(edge_dst, np.int64)
    bat = np.asarray(batch, np.int64)
    x = np.asarray(x, f32)

    deg = np.bincount(dst, minlength=N).astype(np.float64) + 1.0
    dis = 1.0 / np.sqrt(deg)
    cnt = np.maximum(np.bincount(bat, minlength=G), 1).astype(np.float64)

    # globally degree-balanced node->core assignment: sort all nodes by
    # degree, deal them round-robin to cores; per-core position = deal rank.
    # Per-(core,tile) in-degree sums become nearly identical across cores,
    # so the max-over-cores chunk padding almost vanishes.
    grank = np.argsort(-deg, kind="stable")    # node ids, degree-descending
    core_of = np.empty(N, np.int64)
    pos = np.empty(N, np.int64)
    order = np.empty(N, np.int64)      # order[k*NV+j] = node at position j
    core_of[grank] = np.arange(N) % P
    pos[grank] = np.arange(N) // P
    order[core_of * NV + pos] = np.arange(N)
    slot = core_of * NV + pos          # row of node in AllGathered h

    # ---- layer-1 edges (self-loops handled separately), grouped
    #      (core, tile, src-half)
    d_own = core_of[dst]
    d_pos = pos[dst]
    t_of = d_pos // 128
    dloc_v = (d_pos % 128).astype(f32)
    sslot = slot[src]
    is_hi = (sslot >= HALF).astype(np.int64)
    idx_v = (sslot - is_hi * HALF).astype(np.int16)

    key = (d_own * NT + t_of) * 2 + is_hi
    ordr = np.argsort(key, kind="stable")
    idx_s = idx_v[ordr]
    dloc_s = dloc_v[ordr]
    bounds = np.searchsorted(key[ordr], np.arange(P * NT * 2 + 1))
    cnts = np.diff(bounds).reshape(P, NT, 2)
    CH = -(-cnts // 128)               # chunks per (core, tile, half)
    CH = CH.max(axis=0)                # [NT, 2]  uniform across cores

    # batches: stride-interleaved tiles so per-batch work is balanced
    tiles_of_batch = [[t for t in range(NT) if t % NB == b] for b in range(NB)]

    # chunk-column / gather-index layout (shared by all cores)
    # gather order per batch: [lo chunks tile-major][hi chunks tile-major]
    # dloc column order per batch: tile-major, each tile [lo chunks][hi]
    # (so one batched is_equal per tile builds all its selectors)
    batch_meta = []       # per batch: dict(nlo, nhi, icol_lo, icol_hi)
    tile_chunks = {}      # t -> [(half, dloc_col, gather_j)]
    tile_colbase = {}     # t -> first dloc column
    col = 0
    icol = 0
    for b in range(NB):
        nlo = int(sum(CH[t, 0] for t in tiles_of_batch[b]))
        nhi = int(sum(CH[t, 1] for t in tiles_of_batch[b]))
        batch_meta.append(dict(nlo=nlo, nhi=nhi, icol_lo=icol,
                               icol_hi=icol + nlo * 8))
        jlo = jhi = 0
        for t in tiles_of_batch[b]:
            tile_colbase[t] = col
            lst = []
            for _ in range(int(CH[t, 0])):
                lst.append((0, col, jlo))
                col += 1
                jlo += 1
            for _ in range(int(CH[t, 1])):
                lst.append((1, col, jhi))
                col += 1
                jhi += 1
            tile_chunks[t] = lst
        icol += (nlo + nhi) * 8
    NCH = col
    NIDX = NCH * 128
    if os.environ.get("GCN_DEBUG"):
        print(f"NCH={NCH} NIDX={NIDX} real_msgs={cnts.sum(axis=(1,2)).max()}")

    # per-core data arrays
    xT = np.zeros((P, 128, TPAD), bf16)
    disc = np.zeros((P, 128, NT), f32)
    qb = np.zeros((P, TPAD, GP), f32)
    dloc_all = np.full((P, 128, NCH), -1.0, f32)
    idx_flat = np.zeros((P, NIDX), np.int16)

    for k in range(P):
        ok = order[k * NV:(k + 1) * NV]
        # dis[src] prescale folded into x so phase B is a pure matmul
        xT[k, :, :NV] = (x[ok] * dis[ok][:, None]).T.astype(bf16)
        d = np.zeros(TPAD, f32)
        d[:NV] = dis[ok].astype(f32)
        disc[k] = d.reshape(NT, 128).T

    # fill chunk idx / dloc tables
    for b in range(NB):
        m = batch_meta[b]
        for h, base_icol in ((0, m["icol_lo"]), (1, m["icol_hi"])):
            jh = 0
            for t in tiles_of_batch[b]:
                nchunk = int(CH[t, h])
                if nchunk > 0:
                    dcolbase = tile_colbase[t] + (0 if h == 0
                                                  else int(CH[t, 0]))
                    for k in range(P):
                        gi = (k * NT + t) * 2 + h
                        g0, g1 = bounds[gi], bounds[gi + 1]
                        n = g1 - g0
                        fbase = base_icol * 16 + jh * 128
                        idx_flat[k, fbase:fbase + n] = idx_s[g0:g1]
                        pp = np.arange(n) % 128
                        cc = np.arange(n) // 128
                        dloc_all[k, pp, dcolbase + cc] = dloc_s[g0:g1]
                jh += nchunk
    assert idx_flat.min() >= 0 and int(idx_flat.max()) < HALF
    # wrap gather indices: i -> [i % 16, i // 16], replicated to 128 partitions
    idxs = np.tile(
        idx_flat.reshape(P, NIDX // 16, 16).transpose(0, 2, 1), (1, 8, 1)
    ).astype(np.int16)

    # ---- layer-2 Q blocks: qb[core, pos[src], g] += norm/cnt[g]
    #      (self-loops included here)
    e_src = np.concatenate([src, np.arange(N)])
    e_dst = np.concatenate([dst, np.arange(N)])
    g_of = bat[e_dst]
    val = (dis[e_src] * dis[e_dst] / cnt[g_of]).astype(f32)
    np.add.at(qb, (core_of[e_src], pos[e_src], g_of), val)

    iota2d = np.broadcast_to(
        np.arange(128, dtype=f32), (128, 128)).astype(bf16).copy()
    eye128 = np.eye(128, dtype=f32).astype(bf16)
    eye16 = np.eye(16, dtype=f32)

    qb = qb.astype(bf16)
    dloc_bf = dloc_all.astype(bf16)

    W1 = np.ascontiguousarray(np.asarray(W1, f32).astype(bf16))
    W2 = np.ascontiguousarray(np.asarray(W2, f32))
    b1 = np.asarray(b1, f32)
    b2 = np.asarray(b2, f32)
    use_b1 = bool(np.any(b1))
    use_b2 = bool(np.any(b2))

    in_maps = []
    for k in range(P):
        m = {
            "xT": np.ascontiguousarray(xT[k]),
            "qb": np.ascontiguousarray(qb[k]),
            "idxs": np.ascontiguousarray(idxs[k]),
            "dloc": np.ascontiguousarray(dloc_bf[k]),
            "disc": np.ascontiguousarray(disc[k]),
            "w1": W1, "w2": W2,
            "iota": iota2d, "eye128": eye128, "eye16": eye16,
        }
        if use_b1:
            rr = np.zeros((1, TPAD), f32)
            rr[0, :NV] = np.sqrt(deg[order[k * NV:(k + 1) * NV]]).astype(f32)
            m["rdis"] = rr
            m["b1r"] = b1.reshape(1, F)
        if use_b2:
            m["b2r"] = b2.reshape(C, 1)
        in_maps.append(m)

    plan = dict(NCH=NCH, NIDX=NIDX, CH=CH, tiles_of_batch=tiles_of_batch,
                tile_chunks=tile_chunks, tile_colbase=tile_colbase,
                batch_meta=batch_meta, use_b1=use_b1, use_b2=use_b2)
    return plan, in_maps


# ---------------------------------------------------------------- bass build
def _build(plan):
    dt = mybir.dt
    f32, bf16, i16 = dt.float32, dt.bfloat16, dt.int16
    NCH, NIDX = plan["NCH"], plan["NIDX"]
    use_b1, use_b2 = plan["use_b1"], plan["use_b2"]

    nc = bacc.Bacc("TRN2", target_bir_lowering=False, debug=False,
                   num_devices=P, num_swdge_queues=4)
    xT_d = nc.dram_tensor("xT", [128, TPAD], bf16, kind="ExternalInput")
    qb_d = nc.dram_tensor("qb", [TPAD, GP], bf16, kind="ExternalInput")
    idxs_d = nc.dram_tensor("idxs", [128, NIDX // 16], i16, kind="ExternalInput")
    dloc_d = nc.dram_tensor("dloc", [128, NCH], bf16, kind="ExternalInput")
    disc_d = nc.dram_tensor("disc", [128, NT], f32, kind="ExternalInput")
    w1_d = nc.dram_tensor("w1", [F, HID], bf16, kind="ExternalInput")
    w2_d = nc.dram_tensor("w2", [HID, C], f32, kind="ExternalInput")
    iota_d = nc.dram_tensor("iota", [128, 128], bf16, kind="ExternalInput")
    eye128_d = nc.dram_tensor("eye128", [128, 128], bf16, kind="ExternalInput")
    eye_d = nc.dram_tensor("eye16", [16, 16], f32, kind="ExternalInput")
    if use_b1:
        rdis_d = nc.dram_tensor("rdis", [1, TPAD], f32, kind="ExternalInput")
        b1_d = nc.dram_tensor("b1r", [1, F], f32, kind="ExternalInput")
    if use_b2:
        b2_d = nc.dram_tensor("b2r", [C, 1], f32, kind="ExternalInput")
    y_d = nc.dram_tensor("y", [G, C], f32, kind="ExternalOutput")
    shared = bool(int(os.environ.get("GCN_SHARED", "1")))
    # AllGather output in the Shared scratchpad (faster HBM-HBM collective)
    h_full = nc.dram_tensor("h_full_sh", [N, F], bf16, kind="Internal",
                            addr_space="Shared" if shared else "Local")

    with tile.TileContext(nc) as tc:
        cpool = tc.alloc_tile_pool(name="const", bufs=1)
        dram = tc.alloc_tile_pool(name="dram", bufs=1, space="DRAM")

        w1_sb = cpool.tile([F, HID], bf16)
        nc.sync.dma_start(w1_sb[:], w1_d[:, :])
        w2_sb = cpool.tile([HID, C], f32)
        nc.sync.dma_start(w2_sb[:], w2_d[:, :])
        disc_sb = cpool.tile([128, NT], f32)
        nc.sync.dma_start(disc_sb[:], disc_d[:, :])
        iota_sb = cpool.tile([128, 128], bf16)
        nc.sync.dma_start(iota_sb[:], iota_d[:, :])
        eye128_sb = cpool.tile([128, 128], bf16)
        nc.sync.dma_start(eye128_sb[:], eye128_d[:, :])
        eye_sb = cpool.tile([16, 16], f32)
        nc.sync.dma_start(eye_sb[:], eye_d[:, :])
        idxs_sb = cpool.tile([128, NIDX // 16], i16)
        nc.sync.dma_start(idxs_sb[:], idxs_d[:, :])
        dloc_sb = cpool.tile([128, NCH], bf16)
        nc.sync.dma_start(dloc_sb[:], dloc_d[:, :])
        h1_sb = cpool.tile([128, TPAD], bf16)
        hb_sb = cpool.tile([128, TPAD], bf16)   # own h tiles (self-loop seed)
        if use_b1:
            rdis_sb = cpool.tile([1, TPAD], f32)
            nc.sync.dma_start(rdis_sb[:], rdis_d[:, :])
            b1_sb = cpool.tile([1, F], f32)
            nc.sync.dma_start(b1_sb[:], b1_d[:, :])
        if use_b2:
            b2c_sb = cpool.tile([C, 1], f32)
            nc.sync.dma_start(b2c_sb[:], b2_d[:, :])

        h_own = dram.tile([NV, F], bf16)
        ar_in = dram.tile([16, GP], f32)
        ar_out = dram.tile([16, GP], f32)

        # ---------------- phase B: h = (dis*x) @ W1, AllGather
        # dis prescale folded into xT on host; 4 tiles share a PSUM bank so
        # one activation Copy moves 512 columns at a time.
        with (
            tc.tile_pool(name="xw", bufs=1) as xw,
            tc.tile_pool(name="hp", bufs=2, space="PSUM") as hp,
        ):
            xT_sb = xw.tile([128, TPAD], bf16)
            # chunked load so the first matmuls start early
            XC = TPAD // 8
            for ci in range(8):
                nc.sync.dma_start(xT_sb[:, ci * XC:(ci + 1) * XC],
                                  xT_d[:, ci * XC:(ci + 1) * XC])
            for g0 in range(0, NT, 4):
                gn = min(4, NT - g0)
                ps = hp.tile([128, gn * 128], f32)
                for j in range(gn):
                    t = g0 + j
                    nc.tensor.matmul(ps[:, j * 128:(j + 1) * 128],
                                     lhsT=xT_sb[:, t * 128:(t + 1) * 128],
                                     rhs=w1_sb[:], start=True, stop=True)
                nc.scalar.activation(
                    hb_sb[:, g0 * 128:(g0 + gn) * 128], ps[:], AF.Copy)
                for j in range(gn):
                    t = g0 + j
                    rows = min(128, NV - t * 128)
                    if rows > 0:
                        nc.sync.dma_start(
                            h_own[t * 128:t * 128 + rows, :],
                            hb_sb[0:rows, t * 128:(t + 1) * 128])

        nc.gpsimd.collective_compute(
            "AllGather", ALU.bypass, replica_groups=[list(range(P))],
            ins=[h_own[:].opt()], outs=[h_full[:, :].opt()])

        # ---------------- phase C: layer-1 aggregation + layer-2 contraction
        with tc.tile_pool(name="ptp", bufs=1, space="PSUM") as ptp:
            poolT = ptp.tile([128, GP], f32)
            i_l2 = 0
            with (
                tc.tile_pool(name="glo", bufs=4) as glo_p,
                tc.tile_pool(name="ghi", bufs=4) as ghi_p,
                tc.tile_pool(name="selp", bufs=16) as selp,
                tc.tile_pool(name="qp", bufs=6) as qp,
                tc.tile_pool(name="aggp", bufs=7, space="PSUM") as aggp,
            ):
                for b in range(NB):
                    m = plan["batch_meta"][b]
                    nlo, nhi = m["nlo"], m["nhi"]
                    glo = ghi = None
                    # queues map to Q7 core pairs; 4 queues run 4
                    # descriptor generators concurrently
                    if nlo:
                        glo = glo_p.tile([128, nlo, 128], bf16, tag="glo")
                        nc.gpsimd.dma_gather(
                            out_ap=glo[:], in_ap=h_full[0:HALF, :],
                            idxs_ap=idxs_sb[:, m["icol_lo"]:
                                            m["icol_lo"] + nlo * 8],
                            num_idxs=nlo * 128, num_idxs_reg=nlo * 128,
                            elem_size=F, single_packet=False,
                            queue_num=(2 * b) % 4)
                    if nhi:
                        ghi = ghi_p.tile([128, nhi, 128], bf16, tag="ghi")
                        nc.gpsimd.dma_gather(
                            out_ap=ghi[:], in_ap=h_full[HALF:N, :],
                            idxs_ap=idxs_sb[:, m["icol_hi"]:
                                            m["icol_hi"] + nhi * 8],
                            num_idxs=nhi * 128, num_idxs_reg=nhi * 128,
                            elem_size=F, single_packet=False,
                            queue_num=(2 * b + 1) % 4)
                    for t in plan["tiles_of_batch"][b]:
                        chunks = plan["tile_chunks"][t]
                        K = len(chunks)
                        c0 = plan["tile_colbase"][t]
                        ps = aggp.tile([128, 128], f32, tag="agg")
                        # self-loop seed: I @ h_own_tile (PE-ordered with the
                        # accumulating chunk matmuls -- deterministic)
                        nc.tensor.matmul(
                            ps[:], lhsT=eye128_sb[:],
                            rhs=hb_sb[:, t * 128:(t + 1) * 128],
                            start=True, stop=(K == 0 and not use_b1))
                        if use_b1:
                            nc.tensor.matmul(
                                ps[:], lhsT=rdis_sb[0:1, t * 128:(t + 1) * 128],
                                rhs=b1_sb[:], start=False, stop=(K == 0))
                        if K:
                            sel = selp.tile([128, K * 128], bf16, tag="sel")
                            if bool(int(os.environ.get("GCN_SEL3D", "1"))):
                                nc.vector.tensor_tensor(
                                    out=sel[:].rearrange("p (k f) -> p k f",
                                                         k=K),
                                    in0=iota_sb[:].unsqueeze(1).broadcast_to(
                                        [128, K, 128]),
                                    in1=dloc_sb[:, c0:c0 + K].unsqueeze(2)
                                        .broadcast_to([128, K, 128]),
                                    op=ALU.is_equal)
                            else:
                                for ci in range(K):
                                    nc.vector.tensor_tensor(
                                        out=sel[:, ci * 128:(ci + 1) * 128],
                                        in0=iota_sb[:],
                                        in1=dloc_sb[:, c0 + ci:c0 + ci + 1]
                                            .to_broadcast([128, 128]),
                                        op=ALU.is_equal)
                        for ci, (h, dcol, j) in enumerate(chunks):
                            gsrc = ghi if h else glo
                            nc.tensor.matmul(
                                ps[:], lhsT=sel[:, ci * 128:(ci + 1) * 128],
                                rhs=gsrc[:, j, :],
                                start=False, stop=(ci == K - 1))
                        nc.scalar.activation(
                            h1_sb[:, t * 128:(t + 1) * 128], ps[:], AF.Relu,
                            scale=disc_sb[:, t:t + 1])
                        # layer 2: poolT += H1_tile^T-contraction with Q block
                        qt = qp.tile([128, GP], bf16, tag="q")
                        nc.sync.dma_start(
                            qt[:], qb_d[t * 128:(t + 1) * 128, :])
                        nc.tensor.matmul(
                            poolT[:],
                            lhsT=h1_sb[:, t * 128:(t + 1) * 128],
                            rhs=qt[:],
                            start=(i_l2 == 0), stop=(i_l2 == NT - 1))
                        i_l2 += 1

            # project through W2 BEFORE the AllReduce: 256KB -> 32KB payload
            pt_sb = cpool.tile([128, GP], f32)
            nc.scalar.activation(pt_sb[:], poolT[:], AF.Copy)
            with tc.tile_pool(name="w2p", bufs=1, space="PSUM") as w2p:
                out2 = w2p.tile([16, GP], f32)
                nc.tensor.matmul(out2[:], lhsT=w2_sb[:], rhs=pt_sb[:],
                                 start=True, stop=True)
                o2_sb = cpool.tile([16, GP], f32)
                nc.scalar.activation(o2_sb[:], out2[:], AF.Copy)
            nc.sync.dma_start(ar_in[:], o2_sb[:])

        nc.gpsimd.collective_compute(
            "AllReduce", ALU.add, replica_groups=[list(range(P))],
            ins=[ar_in[:].opt()], outs=[ar_out[:].opt()])

        # ---------------- phase D: bias, log_softmax
        with (
            tc.tile_pool(name="fin", bufs=1) as fin,
            tc.tile_pool(name="fps", bufs=2, space="PSUM") as fps,
            tc.tile_pool(name="sm", bufs=4) as smp,
        ):
            logitsT = fin.tile([16, GP], f32)
            nc.sync.dma_start(logitsT[:], ar_out[:])
            if use_b2:
                # bias along classes = per-partition scalar [16, 1]
                nc.vector.tensor_tensor(
                    out=logitsT[:], in0=logitsT[:],
                    in1=b2c_sb[:, 0:1].to_broadcast([16, GP]), op=ALU.add)
            for gt in range(min(GT, -(-G // 128))):
                tp = fps.tile([128, 16], f32, tag="tp")
                nc.tensor.transpose(
                    tp[:], logitsT[:, gt * 128:(gt + 1) * 128], eye_sb[:])
                nmx = smp.tile([128, 1], f32, tag="nmx")
                nc.vector.reduce_max(out=nmx[:], in_=tp[:],
                                     axis=mybir.AxisListType.X, negate=True)
                ex = smp.tile([128, 16], f32, tag="ex")
                nc.scalar.activation(ex[:], tp[:], AF.Exp, bias=nmx[:, 0:1])
                sm = smp.tile([128, 1], f32, tag="sm")
                nc.vector.reduce_sum(out=sm[:], in_=ex[:],
                                     axis=mybir.AxisListType.X)
                lse = smp.tile([128, 1], f32, tag="lse")
                nc.scalar.activation(lse[:], sm[:], AF.Ln)
                res = smp.tile([128, 16], f32, tag="res")
                nc.vector.tensor_scalar(res[:], tp[:], nmx[:, 0:1],
                                        lse[:, 0:1], ALU.add, ALU.subtract)
                rows = min(128, G - gt * 128)
                nc.sync.dma_start(y_d[gt * 128:gt * 128 + rows, :],
                                  res[0:rows, :])
        dram.release()
        cpool.release()
    nc.compile()
    return nc


# ---------------------------------------------------------------- entry
def kernel(x, W1, b1, W2, b2, edge_src, edge_dst, batch):
    global LAST_EXEC_NS, LAST_RESULT
    plan, in_maps = _preprocess(x, W1, b1, W2, b2,
                                edge_src, edge_dst, batch)
    nc = _build(plan)
    trace = bool(int(os.environ.get("GCN_TRACE", "0")))
    kw = {}
    if trace and _install_profile_hook():
        kw = dict(trace=True, trace_cores=[0])
    res = run_bass_kernel_spmd(nc, in_maps, core_ids=list(range(P)), **kw)
    LAST_RESULT = res
    LAST_EXEC_NS = res.exec_time_ns
    return np.ascontiguousarray(res.results[0]["y"].astype(np.float32))
